# revision 66
# baseline (speedup 1.0000x reference)
"""HGT layer Bass kernel for 8 trn2 NeuronCores.

Strategy (dst-parallel, no collectives):
  - Each core owns a contiguous slice of 5000 dst nodes for BOTH relations.
  - Host folds weights:  kv row = [h_src @ (k_w @ bd(rel_att)) * pri/sqrt(dk)
                                   | h_src @ (v_w @ bd(rel_msg)) k-major]
  - Device builds the kv table (DRAM, [N, 256] bf16 per relation) and the
    per-core q tables ([128, NQT*128] bf16, SBUF-resident).
  - Edges sorted by dst, 128-dst blocks; lo/hi passes (int16 idx limit),
    per-block tile caps shared by all cores (SPMD).
  - Per chunk: ONE plain dma_gather of 512B rows [k2|v3] per edge.
  - Per 8-tile group: dbc = partition_broadcast(dstm_flat) (Pool),
    oh2 = is_equal(dbc, iota_col) (DVE 4x), qsel = oh2^T @ q_blk (PE),
    qsel->bf16 (ACT), prod = k2*qsel (DVE 4x), score = bf16 tree reduce
    (DVE 4x), w = exp (ACT), msg = v3*w mid-broadcast (DVE 4x),
    oh = is_equal(iota_row, dstm) per subtile (DVE 4x), scatter
    agg += oh^T @ [msg|w] (PE->PSUM).
  - v3 columns (and a_w rows) permuted k-major so the w-broadcast in msg is
    on a middle dim (keeps the packed last dim -> DVE 4x mode).
  - Finalize after BOTH relations (2 act-table loads total): agg/z, @a_w,
    +h+a_b residual, LayerNorm with Sqrt-only ACT funcs, batched hp/out DMA.
"""

import math
import sys

import numpy as np

sys.path.insert(0, "/opt/trn_rl_repo")

N = 40000
E = 640000
H = 8
DK = 16
D = 128
NCORE = 8
ND = N // NCORE          # 5000 dst nodes per core
NB = (ND + 127) // 128   # 40 blocks (last has 8 dsts)
LO_LIM = 32768
CH_TILES = 16           # tiles per gather chunk (5120 edges)
GRP = 8                  # tiles per compute group
EPS = 1e-5

# v3/a_w permutation: device column j = k*8+h holds original h*16+k
PERM = np.arange(128).reshape(H, DK).T.reshape(-1)  # [k*8+h] -> h*16+k


def _block_diag(m):  # [H, DK, DK] -> [H*DK, H*DK]
    out = np.zeros((H * DK, H * DK), np.float32)
    for h in range(H):
        out[h * DK:(h + 1) * DK, h * DK:(h + 1) * DK] = m[h]
    return out


def _wrap16(a):  # [L] int -> [128, L//16] int16 wrapped+replicated
    arr = np.asarray(a, np.int16).reshape(-1, 16).T  # [16, L/16]
    return np.tile(arr, (8, 1)).copy()


def _prep(inputs):
    """Host-side fold + edge prep. Returns dict of per-core device inputs and
    the static schedule (caps) shared by all cores."""
    f32 = np.float32
    h = [np.asarray(inputs["h_A"], f32), np.asarray(inputs["h_B"], f32)]
    k_w, k_b = np.asarray(inputs["k_w"], f32), np.asarray(inputs["k_b"], f32)
    q_w, q_b = np.asarray(inputs["q_w"], f32), np.asarray(inputs["q_b"], f32)
    v_w, v_b = np.asarray(inputs["v_w"], f32), np.asarray(inputs["v_b"], f32)
    a_w, a_b = np.asarray(inputs["a_w"], f32), np.asarray(inputs["a_b"], f32)
    rel_pri = np.asarray(inputs["rel_pri"], f32)
    rel_att = np.asarray(inputs["rel_att"], f32)
    rel_msg = np.asarray(inputs["rel_msg"], f32)

    P = {}
    P["ln_scale"] = np.asarray(inputs["ln_scale"], f32)
    P["ln_bias"] = np.asarray(inputs["ln_bias"], f32)
    P["ln_trivial"] = [
        bool(np.all(P["ln_scale"][t] == 1.0) and np.all(P["ln_bias"][t] == 0.0))
        for t in range(2)
    ]

    Wkv, bkv, Wkv_sim, bkv_sim = [], [], [], []
    for r in range(2):
        ts = 0 if r == 0 else 1
        scale = np.repeat(rel_pri[r] / math.sqrt(DK), DK)  # [128] per out col
        BDa = _block_diag(rel_att[r])
        BDm = _block_diag(rel_msg[r])
        Wk2 = (k_w[ts] @ BDa) * scale[None, :]
        bk2 = (k_b[ts] @ BDa) * scale
        Wv2 = v_w[ts] @ BDm
        bv2 = v_b[ts] @ BDm
        Wkv_sim.append(np.concatenate([Wk2, Wv2], axis=1))
        bkv_sim.append(np.concatenate([bk2, bv2]))
        Wkv.append(np.concatenate([Wk2, Wv2[:, PERM]], axis=1))  # [128, 256]
        bkv.append(np.concatenate([bk2, bv2[PERM]]))             # [256]
    P["Wkv"], P["bkv"] = Wkv, bkv
    P["Wkv_sim"], P["bkv_sim"] = Wkv_sim, bkv_sim
    P["has_bkv"] = [bool(np.any(b != 0)) for b in bkv]
    P["a_w"] = a_w
    P["a_w_perm"] = [a_w[t][PERM, :] for t in range(2)]
    P["Wq"] = [q_w[0], q_w[1]]
    P["bq"] = [q_b[0], q_b[1]]
    P["has_bq"] = [bool(np.any(b != 0)) for b in q_b]
    P["hT"] = [np.ascontiguousarray(h[t].T) for t in range(2)]  # [128, N]
    P["a_b"] = a_b
    P["h"] = h

    P["iota_row"] = np.tile(np.arange(128, dtype=f32)[None, :], (128, 1))
    S = np.zeros((128, H), f32)
    for hh in range(H):
        S[hh * DK:(hh + 1) * DK, hh] = 1.0
    P["S"] = S
    P["iota_col"] = np.arange(128, dtype=f32)[:, None]

    # per (rel, pass): caps[b] shared across cores, and per-core arrays
    edge = {}
    for r in range(2):
        src = np.asarray(inputs[f"src{r}"], np.int64)
        dst = np.asarray(inputs[f"dst{r}"], np.int64)
        cores = []
        for c in range(NCORE):
            sel = (dst >= c * ND) & (dst < (c + 1) * ND)
            s_c, d_c = src[sel], dst[sel] - c * ND
            order = np.argsort(d_c, kind="stable")
            s_c, d_c = s_c[order], d_c[order]
            blk = d_c // 128
            lo = s_c < LO_LIM
            per = []  # per block: (src_lo, dst_lo, src_hi, dst_hi)
            for b in range(NB):
                m = blk == b
                per.append((s_c[m & lo], d_c[m & lo] - b * 128,
                            s_c[m & ~lo] - LO_LIM, d_c[m & ~lo] - b * 128))
            cores.append(per)
        for p in range(2):  # 0=lo, 1=hi
            caps = []
            for b in range(NB):
                mx = max(len(cores[c][b][2 * p]) for c in range(NCORE))
                caps.append((mx + 127) // 128)
            Lp = sum(caps) * 128
            nch = (Lp // 128 + CH_TILES - 1) // CH_TILES if Lp else 0
            sidx = np.zeros((NCORE, Lp), np.int64)
            dstm = np.full((NCORE, Lp), -1, np.int64)
            for c in range(NCORE):
                off = 0
                for b in range(NB):
                    s_b = cores[c][b][2 * p]
                    d_b = cores[c][b][2 * p + 1]
                    n = len(s_b)
                    sidx[c, off:off + n] = s_b
                    dstm[c, off:off + n] = d_b
                    off += caps[b] * 128
            # static tile schedule: per tile its block; start/stop flags
            tiles = []
            for b in range(NB):
                tiles += [b] * caps[b]
            first = {}
            last = {}
            for t, b in enumerate(tiles):
                if b not in first:
                    first[b] = t
                last[b] = t
            edge[(r, p)] = dict(caps=caps, Lp=Lp, nch=nch, sidx=sidx,
                                dstm=dstm, tiles=tiles, first=first, last=last)
    P["edge"] = edge
    return P


def _build_program(P):
    import concourse.bacc as bacc
    import concourse.mybir as mybir
    from concourse.tile import TileContext
    from contextlib import ExitStack

    f32, bf16, i16 = mybir.dt.float32, mybir.dt.bfloat16, mybir.dt.int16
    AF = mybir.ActivationFunctionType
    OP = mybir.AluOpType

    nc = bacc.Bacc("TRN2")

    # ---- I/O ----
    inp = {}
    def I(name, shape, dt):
        inp[name] = nc.dram_tensor(name, shape, dt, kind="ExternalInput")
        return inp[name]

    hT = [I("hT_A", [D, N], bf16), I("hT_B", [D, N], bf16)]
    hTq = [I("hTq_A", [D, ND], bf16), I("hTq_B", [D, ND], bf16)]
    Wkv_d = [I(f"Wkv{r}", [D, 256], bf16) for r in range(2)]
    bkv_d = [I(f"bkv{r}", [1, 256], bf16) for r in range(2)]
    Wq_d = [I(f"Wq{t}", [D, D], bf16) for t in range(2)]
    bq_d = [I(f"bq{t}", [1, D], bf16) for t in range(2)]
    ones_d = I("ones1", [1, D], bf16)
    aw_d = [I(f"aw{t}", [D, D], bf16) for t in range(2)]
    iota_r_d = I("iota_r", [128, 128], bf16)
    S_d = I("S", [D, H], bf16)
    iota_c_d = I("iota_c", [128, 1], f32)
    hp_d = [I("hp_A", [ND, D], f32), I("hp_B", [ND, D], f32)]
    gb_d = []
    for t in range(2):
        if P["ln_trivial"][t]:
            gb_d.append(None)
        else:
            gb_d.append((I(f"g{t}", [128, D], f32), I(f"bb{t}", [128, D], f32)))
    eidx = {}
    for (r, p), ed in P["edge"].items():
        Lp, nch = ed["Lp"], ed["nch"]
        if Lp == 0:
            continue
        eidx[(r, p)] = (
            I(f"sidx_{r}_{p}", [128, Lp // 16], i16),
            I(f"dstm_{r}_{p}", [128, Lp // 128], f32),
            I(f"dflat_{r}_{p}", [nch, CH_TILES * 128], bf16),
        )
    out_d = nc.dram_tensor("out", [2, ND, D], f32, kind="ExternalOutput")

    # internal DRAM kv tables, split at LO_LIM so lo-pass gathers can
    # start before the hi rows are projected
    kv_lo = [nc.dram_tensor(f"kvlo_{r}", [LO_LIM, 256], bf16)
             for r in range(2)]
    kv_hi = [nc.dram_tensor(f"kvhi_{r}", [N - LO_LIM, 256], bf16)
             for r in range(2)]

    NT = (N + 127) // 128          # 313 node tiles (last width 64)
    NQT = (ND + 127) // 128        # 40 q tiles (last width 8)

    with TileContext(nc) as tc, ExitStack() as ctx:
        const = ctx.enter_context(tc.tile_pool(name="const", bufs=1))
        from concourse.masks import make_identity
        ident_sb = const.tile([128, 128], bf16, tag="ident")
        make_identity(nc, ident_sb[:, :])
        aw_sb = [const.tile([D, D], bf16, tag=f"aw{t}", name=f"aw_sb{t}")
                 for t in range(2)]
        for t in range(2):
            nc.sync.dma_start(out=aw_sb[t][:, :], in_=aw_d[t][:, :])
        iota_r_sb = const.tile([128, 128], bf16, tag="iota_r")
        nc.sync.dma_start(out=iota_r_sb[:, :], in_=iota_r_d[:, :])
        S_sb = const.tile([D, H], bf16, tag="S")
        nc.sync.dma_start(out=S_sb[:, :], in_=S_d[:, :])
        iota_c_sb = const.tile([128, 1], f32, tag="iota_c")
        nc.sync.dma_start(out=iota_c_sb[:, :], in_=iota_c_d[:, :])
        Wkv_sb = [const.tile([D, 256], bf16, tag=f"wkv{r}", name=f"Wkv_sb{r}")
                  for r in range(2)]
        Wq_sb = [const.tile([D, D], bf16, tag=f"wq{t}", name=f"Wq_sb{t}")
                 for t in range(2)]
        for r in range(2):
            nc.sync.dma_start(out=Wkv_sb[r][:, :], in_=Wkv_d[r][:, :])
        for t in range(2):
            nc.sync.dma_start(out=Wq_sb[t][:, :], in_=Wq_d[t][:, :])
        ones_sb = const.tile([1, D], bf16, tag="ones")
        nc.sync.dma_start(out=ones_sb[:, :], in_=ones_d[:, :])
        bias_sb = {}
        for r in range(2):
            if P["has_bkv"][r]:
                b_sb = const.tile([1, 256], bf16, tag=f"bkv{r}")
                nc.sync.dma_start(out=b_sb[:, :], in_=bkv_d[r][:, :])
                bias_sb[("kv", r)] = b_sb
        for t in range(2):
            if P["has_bq"][t]:
                b_sb = const.tile([1, D], bf16, tag=f"bq{t}")
                nc.sync.dma_start(out=b_sb[:, :], in_=bq_d[t][:, :])
                bias_sb[("q", t)] = b_sb
        gb_sb = []
        for t in range(2):
            if gb_d[t] is None:
                gb_sb.append(None)
            else:
                g_sb = const.tile([128, D], f32, tag=f"g{t}")
                b2_sb = const.tile([128, D], f32, tag=f"b{t}")
                nc.sync.dma_start(out=g_sb[:, :], in_=gb_d[t][0][:, :])
                nc.sync.dma_start(out=b2_sb[:, :], in_=gb_d[t][1][:, :])
                gb_sb.append((g_sb, b2_sb))
        # SBUF-resident q tables [128, NQT*128] bf16, partition = dst%128
        q_sb = [const.tile([128, NQT * 128], bf16, tag=f"q{t}", name=f"q_sb{t}")
                for t in range(2)]
        # zero the tail tile's unused partitions (avoid NaN*0 in PE)
        for t in range(2):
            nc.vector.memset(q_sb[t][:, (NQT - 1) * 128: NQT * 128], 0.0)

        # pools
        ppool = ctx.enter_context(tc.tile_pool(name="proj", bufs=2))
        pps = ctx.enter_context(tc.tile_pool(name="projps", bufs=2,
                                             space="PSUM"))
        epool = ctx.enter_context(tc.tile_pool(name="edge", bufs=5))
        dpool = ctx.enter_context(tc.tile_pool(name="dflat", bufs=2))
        work = ctx.enter_context(tc.tile_pool(name="work", bufs=3))
        hpool = ctx.enter_context(tc.tile_pool(name="whead", bufs=2))
        wtail = ctx.enter_context(tc.tile_pool(name="wtail", bufs=3))
        idxp = ctx.enter_context(tc.tile_pool(name="idx", bufs=2))
        ps_q = ctx.enter_context(tc.tile_pool(name="ps_q", bufs=1,
                                              space="PSUM"))
        ps_v = ctx.enter_context(tc.tile_pool(name="ps_v", bufs=1,
                                              space="PSUM"))
        ps_s = ctx.enter_context(tc.tile_pool(name="ps_s", bufs=1,
                                              space="PSUM"))
        ps_a = ctx.enter_context(tc.tile_pool(name="ps_a", bufs=2,
                                              space="PSUM"))
        aggp = ctx.enter_context(tc.tile_pool(name="agg", bufs=1))
        fin = ctx.enter_context(tc.tile_pool(name="fin", bufs=4))
        fin2 = ctx.enter_context(tc.tile_pool(name="fin2", bufs=2))

        # ---------- Phase P: kv tables (DRAM) + q tables (SBUF) ----------
        copy_flip = [0]
        proj_interleaved = [False]

        def psum_copy(dst_ap, src_ap):
            # NOTE: Pool/gpsimd cannot read PSUM (BIR verifier rejects it),
            # so staging alternates ACT / DVE.
            if copy_flip[0] % 3 == 2:
                nc.vector.tensor_copy(out=dst_ap, in_=src_ap)
            else:
                nc.scalar.copy(out=dst_ap, in_=src_ap)
            copy_flip[0] += 1

        def kv_dst(r, j, w):
            # DRAM row range for node tile j (width w rows)
            if j * 128 < LO_LIM:
                return kv_lo[r][j * 128: j * 128 + w, :]
            o = j * 128 - LO_LIM
            return kv_hi[r][o: o + w, :]

        def proj_kv(r):
            ts = 0 if r == 0 else 1
            bias = bias_sb.get(("kv", r))
            gi = 0
            # hi-table node tiles first: the (small) hi edge pass runs while
            # the lo table is still being built. LO_LIM=256*128 is 8-aligned,
            # so no chunk straddles the lo/hi table boundary.
            j0_hi = LO_LIM // 128
            for j0 in list(range(j0_hi, NT, 8)) + list(range(0, j0_hi, 8)):
                jn = min(8, NT - j0)
                wtot = min(8 * 128, N - j0 * 128)
                ht = ppool.tile([D, 1024], bf16, tag="htkv")
                nc.sync.dma_start(out=ht[:, 0:wtot],
                                  in_=hT[ts][:, j0 * 128: j0 * 128 + wtot])
                stage = ppool.tile([128, 2048], bf16, tag="stage")
                for j1 in range(0, jn, 2):
                    j2 = min(2, jn - j1)
                    ps = pps.tile([128, 512], f32, tag="pps",
                                  name=f"pps{gi % 2}")
                    gi += 1
                    for jj in range(j2):
                        w = min(128, N - (j0 + j1 + jj) * 128)
                        o = jj * 256
                        c = (j1 + jj) * 128
                        if bias is not None:
                            nc.tensor.matmul(
                                out=ps[0:w, o:o + 256], lhsT=ones_sb[:, 0:w],
                                rhs=bias[:, :], start=True, stop=False)
                            nc.tensor.matmul(
                                out=ps[0:w, o:o + 256], lhsT=ht[:, c:c + w],
                                rhs=Wkv_sb[r][:, :], start=False, stop=True)
                        else:
                            nc.tensor.matmul(
                                out=ps[0:w, o:o + 256], lhsT=ht[:, c:c + w],
                                rhs=Wkv_sb[r][:, :], start=True, stop=True)
                    w2 = min(2 * 128, N - (j0 + j1) * 128)
                    if w2 >= 256:
                        psum_copy(stage[:, j1 * 256:(j1 + 2) * 256], ps[:, :])
                    else:
                        psum_copy(stage[0:w2, j1 * 256: j1 * 256 + 256],
                                  ps[0:w2, 0:256])
                if wtot == jn * 128 and jn % 2 == 0:
                    dst = kv_dst(r, j0, wtot).rearrange(
                        "(jj p) f -> p jj f", p=128)
                    nc.sync.dma_start(
                        out=dst,
                        in_=stage[:, 0:jn * 256].rearrange(
                            "p (jj f) -> p jj f", f=256))
                else:
                    for jj in range(jn):
                        w = min(128, N - (j0 + jj) * 128)
                        nc.sync.dma_start(
                            out=kv_dst(r, j0 + jj, w),
                            in_=stage[0:w, jj * 256: jj * 256 + 256])
                yield

        def proj_q(t):
            bias = bias_sb.get(("q", t))
            gi = 0
            for j0 in range(0, NQT, 8):
                jn = min(8, NQT - j0)
                wtot = min(8 * 128, ND - j0 * 128)
                ht = ppool.tile([D, 1024], bf16, tag="htq")
                nc.sync.dma_start(out=ht[:, 0:wtot],
                                  in_=hTq[t][:, j0 * 128: j0 * 128 + wtot])
                for j1 in range(0, jn, 2):
                    j2 = min(2, jn - j1)
                    ps = pps.tile([128, 512], f32, tag="pps",
                                  name=f"ppsq{gi % 2}")
                    gi += 1
                    for jj in range(j2):
                        w = min(128, ND - (j0 + j1 + jj) * 128)
                        o = jj * 256
                        c = (j1 + jj) * 128
                        if bias is not None:
                            nc.tensor.matmul(
                                out=ps[0:w, o:o + 128], lhsT=ones_sb[:, 0:w],
                                rhs=bias[:, :], start=True, stop=False)
                            nc.tensor.matmul(
                                out=ps[0:w, o:o + 128], lhsT=ht[:, c:c + w],
                                rhs=Wq_sb[t][:, :], start=False, stop=True)
                        else:
                            nc.tensor.matmul(
                                out=ps[0:w, o:o + 128], lhsT=ht[:, c:c + w],
                                rhs=Wq_sb[t][:, :], start=True, stop=True)
                        jg = j0 + j1 + jj
                        nc.scalar.copy(
                            out=q_sb[t][0:w, jg * 128: jg * 128 + 128],
                            in_=ps[0:w, o:o + 128])
                yield

        # emission: q(1) + kv(0) first (edge r0 needs them); q(0)+kv(1)
        # groups are interleaved into the r0 edge loop below.
        for _ in proj_q(1):
            pass
        for _ in proj_kv(0):
            pass
        proj_interleaved[0] = True
        rest_proj = [proj_q(0), proj_kv(1)]

        def emit_proj(n):
            k = 0
            while rest_proj and k < n:
                try:
                    next(rest_proj[0])
                    k += 1
                except StopIteration:
                    rest_proj.pop(0)

        # ---------- finalize (per relation; Sqrt-only ACT funcs) ----------
        def finalize(r):
            # generator: yields after each 4-block group so the caller can
            # interleave emission with other work (filler for PE gaps).
            td = 1 if r == 0 else 0
            agg_sb = agg_sbs[r]
            present = presents[r]
            blocks = [b for b in range(NB) if b in present]
            i = 0
            while i < len(blocks):
                grp = [blocks[i]]
                while (len(grp) < 4 and i + len(grp) < len(blocks)
                       and blocks[i + len(grp)] == grp[0] + len(grp)
                       and (grp[0] + len(grp) + 1) * 128 <= ND):
                    grp.append(blocks[i + len(grp)])
                nb_ = len(grp)
                b0 = grp[0]
                full = (b0 + nb_) * 128 <= ND
                hpb = fin.tile([128, 4, 128], f32, tag="hpb")
                y4 = fin.tile([128, 4, 128], f32, tag="y4")
                if full:
                    nc.sync.dma_start(
                        out=hpb[:, 0:nb_, :],
                        in_=hp_d[td][b0 * 128:(b0 + nb_) * 128, :].rearrange(
                            "(jj p) f -> p jj f", p=128))
                else:
                    w = ND - b0 * 128
                    nc.sync.dma_start(
                        out=hpb[0:w, 0:1, :],
                        in_=hp_d[td][b0 * 128: b0 * 128 + w, :].rearrange(
                            "(jj p) f -> p jj f", p=w))
                agg3 = agg_sb[:, :].rearrange("p (b c) -> p b c", c=136)
                zc4 = fin.tile([128, 4, 8], f32, tag="zc")
                nc.vector.tensor_scalar(
                    out=zc4[:, 0:nb_, :], in0=agg3[:, b0:b0 + nb_, 128:136],
                    scalar1=1e-30, scalar2=None, op0=OP.max)
                rz4 = fin.tile([128, 4, 8], f32, tag="rz")
                nc.vector.reciprocal(out=rz4[:, 0:nb_, :],
                                     in_=zc4[:, 0:nb_, :])
                x2g = fin2.tile([128, 4, 128], f32, tag="x2")
                for j, b in enumerate(grp):
                    w = min(128, ND - b * 128)
                    x = fin.tile([128, 128], bf16, tag="x")
                    nc.vector.tensor_tensor(
                        out=x[:, :].rearrange("p (k h) -> p k h", h=8),
                        in0=agg3[:, b, 0:128].rearrange(
                            "p (k h) -> p k h", h=8),
                        in1=rz4[:, j, :].unsqueeze(1).broadcast_to(
                            [128, 16, 8]),
                        op=OP.mult)
                    psT = pps.tile([128, 128], bf16, tag="pps", name="psT")
                    nc.tensor.transpose(out=psT[:, 0:w], in_=x[0:w, :],
                                        identity=ident_sb[0:w, 0:w])
                    aT = fin.tile([128, 128], bf16, tag="aT")
                    nc.scalar.copy(out=aT[:, 0:w], in_=psT[:, 0:w])
                    psO = pps.tile([128, 128], f32, tag="pps", name="psO")
                    nc.tensor.matmul(out=psO[0:w, :], lhsT=aT[:, 0:w],
                                     rhs=aw_sb[td][:, :], start=True,
                                     stop=True)
                    nc.vector.tensor_tensor(out=x2g[0:w, j, :],
                                            in0=psO[0:w, :],
                                            in1=hpb[0:w, j, :], op=OP.add)
                    st6 = fin.tile([128, 6], f32, tag="st6")
                    nc.vector.bn_stats(out=st6[0:w, :], in_=x2g[0:w, j, :])
                    st2 = fin.tile([128, 2], f32, tag="st2")
                    nc.vector.bn_aggr(out=st2[0:w, :], in_=st6[0:w, :])
                    ve = fin.tile([128, 1], f32, tag="ve")
                    nc.vector.tensor_scalar(
                        out=ve[0:w, :], in0=st2[0:w, 1:2],
                        scalar1=EPS, scalar2=None, op0=OP.add)
                    iv = fin.tile([128, 1], f32, tag="iv")
                    nc.vector.reciprocal(out=iv[0:w, :], in_=ve[0:w, :])
                    rstd = fin.tile([128, 1], f32, tag="rstd")
                    nc.scalar.activation(out=rstd[0:w, :], in_=iv[0:w, :],
                                         func=AF.Sqrt)
                    nmean = fin.tile([128, 1], f32, tag="nmean")
                    nc.vector.tensor_scalar(
                        out=nmean[0:w, :], in0=st2[0:w, 0:1],
                        scalar1=-1.0, scalar2=None, op0=OP.mult)
                    nc.vector.tensor_scalar(
                        out=y4[0:w, j, :], in0=x2g[0:w, j, :],
                        scalar1=nmean[0:w, :], scalar2=rstd[0:w, :],
                        op0=OP.add, op1=OP.mult)
                    if gb_sb[td] is not None:
                        g_sb, b2_sb = gb_sb[td]
                        nc.vector.tensor_tensor(
                            out=y4[0:w, j, :], in0=y4[0:w, j, :],
                            in1=g_sb[0:w, :], op=OP.mult)
                        nc.vector.tensor_tensor(
                            out=y4[0:w, j, :], in0=y4[0:w, j, :],
                            in1=b2_sb[0:w, :], op=OP.add)
                if full:
                    nc.sync.dma_start(
                        out=out_d[td, b0 * 128:(b0 + nb_) * 128, :].rearrange(
                            "(jj p) f -> p jj f", p=128),
                        in_=y4[:, 0:nb_, :])
                else:
                    w = ND - b0 * 128
                    nc.sync.dma_start(
                        out=out_d[td, b0 * 128: b0 * 128 + w, :],
                        in_=y4[0:w, 0, :])
                i += nb_
                yield

        idx_cache = {}

        def load_idx(r, p):
            if (r, p) in idx_cache:
                return idx_cache[(r, p)]
            Lp = P["edge"][(r, p)]["Lp"]
            sidx_d, dstm_d, _ = eidx[(r, p)]
            sidx_sb = idxp.tile([128, Lp // 16], i16, tag="sidx",
                                name=f"sidx{r}{p}")
            dstm_sb = idxp.tile([128, Lp // 128], f32, tag="dstm",
                                name=f"dstm{r}{p}")
            nc.sync.dma_start(out=sidx_sb[:, :], in_=sidx_d[:, :])
            nc.sync.dma_start(out=dstm_sb[:, :], in_=dstm_d[:, :])
            idx_cache[(r, p)] = (sidx_sb, dstm_sb)
            return idx_cache[(r, p)]

        # ---------- Phase E ----------
        agg_sbs = []
        presents = []
        fin_gen = [None]
        for r in range(2):
            td = 1 if r == 0 else 0
            agg_sb = aggp.tile([128, NB * 136], f32, tag="aggsb",
                               name=f"aggsb{r}")
            agg_sbs.append(agg_sb)
            present = set()
            presents.append(present)
            for p in (1, 0):  # hi pass first (its table is built first)
                ed = P["edge"][(r, p)]
                Lp, nch = ed["Lp"], ed["nch"]
                if Lp == 0:
                    continue
                ntiles = Lp // 128
                tiles, first, last = ed["tiles"], ed["first"], ed["last"]
                dflat_d = eidx[(r, p)][2]
                sidx_sb, dstm_sb = load_idx(r, p)

                kv_src = kv_hi[r][:, :] if p == 1 else kv_lo[r][:, :]

                agg_tile = {}  # live block -> psum tile

                for ci in range(nch):
                    g0 = ci * CH_TILES
                    gn = min(CH_TILES, ntiles - g0)
                    G = gn * 128
                    vb = epool.tile([128, 2, G], bf16, tag="vb")
                    nc.gpsimd.dma_gather(
                        vb[:, :, :], kv_src,
                        sidx_sb[:, g0 * 8: g0 * 8 + G // 16],
                        G, G, 256, transpose=True, single_packet=False)
                    dflat = dpool.tile([1, CH_TILES * 128], bf16, tag="dflat")
                    nc.sync.dma_start(out=dflat[:, 0:G],
                                      in_=dflat_d[ci: ci + 1, 0:G])
                    if r == 0:
                        emit_proj(1)

                    # chunk-level one-hot: dbc[p, x] = dstm_flat[x] for all
                    # partitions p, then oh2 = (dbc == iota_col); amortizes
                    # the Pool/DVE per-op overhead over the whole chunk.
                    dbc = hpool.tile([128, CH_TILES * 128], bf16, tag="dbc")
                    nc.gpsimd.partition_broadcast(
                        dbc[:, 0:G], dflat[0:1, 0:G])
                    oh2c = hpool.tile([128, CH_TILES * 128], bf16, tag="oh2")
                    nc.vector.tensor_scalar(
                        out=oh2c[:, 0:G], in0=dbc[:, 0:G],
                        scalar1=iota_c_sb[:, :], scalar2=None,
                        op0=OP.is_equal)

                    # chunk-level rhs [msg|w] and score tiles; one Exp per
                    # chunk (vs per group) amortizes the ACT access overhead.
                    rhs_c = wtail.tile([128, CH_TILES, 136], bf16, tag="rhs")
                    scps = ps_s.tile([128, CH_TILES, 8], f32, tag="scps")

                    # pass A (per group): q-select, prod, score
                    for s0 in range(0, gn, GRP):
                        B = min(GRP, gn - s0)
                        gt = g0 + s0  # global tile idx of group start
                        BW = B * 128
                        oh2 = oh2c[:, s0 * 128:(s0 + B) * 128]
                        # qselT = q_blk^T-select: per same-block run
                        qselT = ps_q.tile([128, GRP * 128], f32, tag="qselT")
                        s = 0
                        while s < B:
                            b = tiles[gt + s]
                            rl = 1
                            while (s + rl < B and tiles[gt + s + rl] == b
                                   and rl < 4):
                                rl += 1
                            nc.tensor.matmul(
                                out=qselT[:, s * 128:(s + rl) * 128],
                                lhsT=q_sb[td][:, b * 128: b * 128 + 128],
                                rhs=oh2[:, s * 128:(s + rl) * 128],
                                start=True, stop=True)
                            s += rl
                        qbfT = work.tile([128, GRP * 128], bf16, tag="qbf")
                        nc.scalar.copy(out=qbfT[:, 0:BW], in_=qselT[:, 0:BW])
                        # prodT[d, e] = k2T * qselT  (2x: bf16 SBUF)
                        prodT = work.tile([128, GRP * 128], bf16, tag="prod")
                        nc.vector.tensor_tensor(
                            out=prodT[:, 0:BW],
                            in0=vb[:, 0, s0 * 128: s0 * 128 + BW],
                            in1=qbfT[:, 0:BW], op=OP.mult)
                        # score[e, h] per subtile via PE S-matmul
                        for s in range(B):
                            nc.tensor.matmul(
                                out=scps[:, s0 + s, :],
                                lhsT=prodT[:, s * 128:(s + 1) * 128],
                                rhs=S_sb[:, :], start=True, stop=True)
                    nc.scalar.activation(
                        out=rhs_c[:, 0:gn, 128:136], in_=scps[:, 0:gn, :],
                        func=AF.Exp)

                    # pass B (per group): v3 transpose, msg, scatter
                    for s0 in range(0, gn, GRP):
                        B = min(GRP, gn - s0)
                        gt = g0 + s0
                        # v3 back to plain layout via PE transpose (k-major)
                        v3p = ps_v.tile([128, GRP, 128], bf16, tag="v3p")
                        for s in range(B):
                            nc.tensor.transpose(
                                out=v3p[:, s, :],
                                in_=vb[:, 1, (s0 + s) * 128:(s0 + s + 1) * 128],
                                identity=ident_sb[:, :])
                        # msg = v3perm * w  (w broadcast on middle dim -> 2x)
                        nc.vector.tensor_tensor(
                            out=rhs_c[:, s0:s0 + B, 0:128].rearrange(
                                "p b (k h) -> p b k h", h=8),
                            in0=v3p[:, 0:B, :].rearrange(
                                "p b (k h) -> p b k h", h=8),
                            in1=rhs_c[:, s0:s0 + B, 128:136].unsqueeze(
                                2).broadcast_to([128, B, 16, 8]),
                            op=OP.mult)
                        # onehot [e, dst] per subtile (4x tensor_scalar)
                        oh = wtail.tile([128, GRP, 128], bf16, tag="oh")
                        for s in range(B):
                            nc.vector.tensor_scalar(
                                out=oh[:, s, :], in0=iota_r_sb[:, :],
                                scalar1=dstm_sb[:, gt + s: gt + s + 1],
                                scalar2=None, op0=OP.is_equal)
                        for s in range(B):
                            t_ = gt + s
                            b = tiles[t_]
                            if b not in agg_tile:
                                agg_tile[b] = ps_a.tile(
                                    [128, 136], f32, tag="psagg",
                                    name=f"psagg{b}")
                            nc.tensor.matmul(
                                out=agg_tile[b][:, :], lhsT=oh[:, s, :],
                                rhs=rhs_c[:, s0 + s, :],
                                start=(t_ == first[b]), stop=(t_ == last[b]))
                            if t_ == last[b]:
                                sl = agg_sb[:, b * 136:(b + 1) * 136]
                                if b not in present:
                                    nc.scalar.copy(out=sl,
                                                   in_=agg_tile[b][:, :])
                                    present.add(b)
                                else:
                                    nc.vector.tensor_tensor(
                                        out=sl, in0=sl, in1=agg_tile[b][:, :],
                                        op=OP.add)
                                del agg_tile[b]

            if r == 0:
                emit_proj(10000)
                for p2 in range(2):
                    if P["edge"][(1, p2)]["Lp"]:
                        load_idx(1, p2)
                fin_gen[0] = finalize(0)
        if fin_gen[0] is not None:
            for _ in fin_gen[0]:
                pass
        for _ in finalize(1):
            pass


    nc.compile()
    return nc, inp


LAST_EXEC_NS = None
LAST_TRACE = None


def kernel(**inputs):
    from concourse.bass_utils import run_bass_kernel_spmd

    P = _prep(inputs)
    nc, _ = _build_program(P)

    import ml_dtypes
    bf16 = ml_dtypes.bfloat16

    in_maps = []
    for c in range(NCORE):
        m = {
            "hT_A": P["hT"][0].astype(bf16),
            "hT_B": P["hT"][1].astype(bf16),
            "hTq_A": np.ascontiguousarray(
                P["hT"][0][:, c * ND:(c + 1) * ND]).astype(bf16),
            "hTq_B": np.ascontiguousarray(
                P["hT"][1][:, c * ND:(c + 1) * ND]).astype(bf16),
            "ones1": np.ones((1, D), bf16),
            "iota_r": P["iota_row"].astype(bf16),
            "S": P["S"].astype(bf16),
            "iota_c": P["iota_col"].astype(np.float32),
            "hp_A": (P["h"][0][c * ND:(c + 1) * ND] + P["a_b"][0][None, :]
                     ).astype(np.float32),
            "hp_B": (P["h"][1][c * ND:(c + 1) * ND] + P["a_b"][1][None, :]
                     ).astype(np.float32),
        }
        for r in range(2):
            m[f"Wkv{r}"] = P["Wkv"][r].astype(bf16)
            m[f"bkv{r}"] = P["bkv"][r][None, :].astype(bf16)
        for t in range(2):
            m[f"Wq{t}"] = P["Wq"][t].astype(bf16)
            m[f"bq{t}"] = P["bq"][t][None, :].astype(bf16)
            m[f"aw{t}"] = P["a_w_perm"][t].astype(bf16)
            if not P["ln_trivial"][t]:
                m[f"g{t}"] = np.tile(P["ln_scale"][t][None, :], (128, 1)
                                     ).astype(np.float32)
                m[f"bb{t}"] = np.tile(P["ln_bias"][t][None, :], (128, 1)
                                      ).astype(np.float32)
        for (r, p), ed in P["edge"].items():
            if ed["Lp"] == 0:
                continue
            m[f"sidx_{r}_{p}"] = _wrap16(ed["sidx"][c])
            m[f"dstm_{r}_{p}"] = np.ascontiguousarray(
                ed["dstm"][c].reshape(-1, 128).T).astype(np.float32)
            flat = np.full((ed["nch"], CH_TILES * 128), -1.0, np.float32)
            flat.reshape(-1)[:ed["Lp"]] = ed["dstm"][c]
            m[f"dflat_{r}_{p}"] = flat.astype(bf16)
        in_maps.append(m)

    import os
    kw = {}
    if os.environ.get("BASS_TRACE"):
        kw = dict(trace=True, tmpdir=os.environ.get("BASS_TRACE_DIR") or None)
    res = run_bass_kernel_spmd(nc, in_maps, list(range(NCORE)), **kw)
    global LAST_EXEC_NS, LAST_TRACE
    LAST_EXEC_NS = res.exec_time_ns
    LAST_TRACE = res.instructions_and_trace
    outs = res.results
    full = np.zeros((2, N, D), np.float32)
    for c in range(NCORE):
        o = np.asarray(outs[c]["out"])
        full[0, c * ND:(c + 1) * ND] = o[0]
        full[1, c * ND:(c + 1) * ND] = o[1]
    return full


def numpy_sim(**inputs):
    """Numpy simulation of the device algorithm (w/ bf16 quantization and the
    bf16 score tree) for fast correctness validation of the host prep."""
    import ml_dtypes
    bf16 = ml_dtypes.bfloat16

    def q(x):
        return np.asarray(x).astype(bf16).astype(np.float32)

    P = _prep(inputs)
    full = np.zeros((2, N, D), np.float32)
    for c in range(NCORE):
        for r in range(2):
            td = 1 if r == 0 else 0
            ts = 0 if r == 0 else 1
            hq = q(P["hT"][ts].T)
            kv = hq @ q(P["Wkv_sim"][r]) + P["bkv_sim"][r]
            k2 = q(kv[:, :128])
            v3 = q(kv[:, 128:])
            qq = q(q(P["hT"][td].T[c * ND:(c + 1) * ND]) @ q(P["Wq"][td])
                   + P["bq"][td])
            agg = np.zeros((ND, 136), np.float32)
            for p in range(2):
                ed = P["edge"][(r, p)]
                if ed["Lp"] == 0:
                    continue
                sidx = ed["sidx"][c].astype(np.int64) + (LO_LIM if p else 0)
                dstm = ed["dstm"][c]
                caps = ed["caps"]
                off = 0
                for b in range(NB):
                    L = caps[b] * 128
                    sl = slice(off, off + L)
                    valid = dstm[sl] >= 0
                    qs = np.where(valid[:, None],
                                  qq[np.clip(b * 128 + dstm[sl], 0, ND - 1)],
                                  0.0)
                    tt = q(q(k2[sidx[sl]]) * q(qs)).reshape(L, H, 16)
                    score = tt.sum(-1)
                    w = q(np.exp(score).astype(np.float32))
                    msg = q(v3[sidx[sl]].reshape(L, H, DK)
                            * w[:, :, None]).reshape(L, 128)
                    d_glob = b * 128 + dstm[sl]
                    for i in np.nonzero(valid)[0]:
                        agg[d_glob[i], :128] += msg[i]
                        agg[d_glob[i], 128:] += w[i]
                    off += L
            z = np.maximum(agg[:, 128:], 1e-30)
            aggn = q((agg[:, :128].reshape(ND, H, DK)
                      / z[:, :, None]).reshape(ND, 128))
            x = aggn @ q(P["a_w"][td])
            x = x + P["h"][td][c * ND:(c + 1) * ND] + P["a_b"][td][None, :]
            mu = x.mean(1, keepdims=True)
            var = x.var(1, keepdims=True)
            y = (x - mu) / np.sqrt(var + EPS)
            y = y * P["ln_scale"][td][None, :] + P["ln_bias"][td][None, :]
            full[td, c * ND:(c + 1) * ND] = y
    return full

